# revision 12
# baseline (speedup 1.0000x reference)
"""MLKV linear-attention decode step (gated delta rule) on 8 TRN2 NeuronCores.

Tensor-parallel over heads: core r owns k-heads {2r, 2r+1} and v-heads
{4r..4r+3}.  Each core computes its slice of the qkv/z/b/a matvecs, the causal
conv update, the gated delta-rule recurrence, the gated RMSNorm, and a partial
W_out matvec (contraction over its 512 output channels).  The host sums the 8
hidden partials (the "all-reduce") and reassembles conv/rec state slices.

All weight matrices are pre-transposed on the host so the contraction dim (the
hidden dim) lands on SBUF partitions; the 2048-dim activation vector is the
matmul stationary operand (128x1 per chunk) and the weight tiles stream as the
moving operand at N=512.  This makes the kernel DMA-bound (~17 MB/core).
"""

import sys

for _p in ("/opt/trn_rl_repo", "/opt/pypackages"):
    if _p not in sys.path:
        sys.path.append(_p)

from contextlib import ExitStack

import numpy as np

import concourse.bass as bass
import concourse.tile as tile
from concourse import bacc
from concourse import mybir
from concourse.bass_utils import run_bass_kernel_spmd
from concourse.masks import make_identity

F32 = mybir.dt.float32

NCORES = 8
H = 2048
NCHUNK = 16          # 2048 / 128 contraction chunks
P = 128
NH = 4               # v-heads per core
NKH = 2              # k-heads per core
KCONV = 4
RQK = float(1.0 / np.sqrt(128.0))
EPS = 1e-6
ROWS_PER_CORE = 1024  # 256 q + 256 k + 512 v conv channels


def _core_rows(r: int) -> np.ndarray:
    """Global W_qkv/conv-channel rows owned by core r, in group order
    [q0 q1 k0 k1 v0 v1 v2 v3] x 128."""
    q = np.arange(256 * r, 256 * r + 256)
    k = 2048 + np.arange(256 * r, 256 * r + 256)
    v = 4096 + np.arange(512 * r, 512 * r + 512)
    return np.concatenate([q, k, v])


def hs(h):
    """Free-dim slice for head h in a [1, 512] row tile."""
    return slice(h * P, (h + 1) * P)


def _build_body(ctx, tc, d, pools):
    nc = tc.nc
    wpool, cpool, spool, pp, pp2 = pools

    # ---- input DMAs --------------------------------------------------------
    # Small inputs ride SWDGE (gpsimd) so they don't delay the big HWDGE
    # weight stream; HWDGE drains FIFO in issue order: wq -> wz -> wba -> wo.
    hT = cpool.tile([P, NCHUNK], F32)
    nc.gpsimd.dma_start(out=hT, in_=d["hT"])
    cs = cpool.tile([P, 8, KCONV], F32)
    nc.gpsimd.dma_start(out=cs, in_=d["cs"].rearrange("p (g k) -> p g k", k=KCONV))
    cw = cpool.tile([P, 8, KCONV], F32)
    nc.gpsimd.dma_start(out=cw, in_=d["cw"].rearrange("p (g k) -> p g k", k=KCONV))
    rec = cpool.tile([P, NH, P], F32)
    nc.gpsimd.dma_start(out=rec, in_=d["rec"].rearrange("h k d -> k h d"))
    dtb = cpool.tile([1, 8], F32)
    nc.gpsimd.dma_start(out=dtb, in_=d["dtba"])
    nw4 = cpool.tile([1, 512], F32)
    nc.gpsimd.dma_start(out=nw4, in_=d["nw4"])
    qksc = cpool.tile([1, 4], F32)
    nc.gpsimd.dma_start(out=qksc, in_=d["qksc"])

    wq = []
    for i in range(4):
        t = wpool.tile([P, 4, 1024], F32, tag=f"wq{i}")
        nc.sync.dma_start(
            out=t,
            in_=d["wqkvT"][i * 512 : (i + 1) * 512, :].rearrange(
                "(c p) r -> p c r", p=P
            ),
        )
        wq.append(t)
    wba = wpool.tile([P, NCHUNK, 8], F32)
    nc.sync.dma_start(out=wba, in_=d["wbaT"].rearrange("p (c j) -> p c j", j=8))
    wz = []
    for i in range(2):
        t = wpool.tile([P, 8, 512], F32, tag=f"wz{i}")
        nc.sync.dma_start(
            out=t,
            in_=d["wzT"][i * 1024 : (i + 1) * 1024, :].rearrange(
                "(c p) r -> p c r", p=P
            ),
        )
        wz.append(t)
    wo = []
    for oc in range(4):
        t = wpool.tile([P, 2048], F32, tag=f"wo{oc}")
        nc.sync.dma_start(out=t, in_=d["woT"][oc * P : (oc + 1) * P, :])
        wo.append(t)

    # ---- constants built on-chip ------------------------------------------
    id128 = cpool.tile([P, P], F32)
    make_identity(nc, id128)
    ones_col = cpool.tile([P, 1], F32)
    nc.gpsimd.memset(ones_col, 1.0)
    ones_row = cpool.tile([1, P], F32)
    nc.gpsimd.memset(ones_row, 1.0)
    one1 = cpool.tile([1, 1], F32)
    nc.gpsimd.memset(one1, 1.0)
    eps1 = cpool.tile([1, 1], F32)
    nc.gpsimd.memset(eps1, EPS)

    # ---- phase 1: matvecs against h ---------------------------------------
    mixedA = pp.tile([1, 512], F32, tag="bankA")   # q0 q1 k0 k1 channels
    mixedB = pp.tile([1, 512], F32, tag="bankB")   # v0..v3 channels
    ba_ps = pp.tile([1, 8], F32, tag="bankD")
    for n in range(NCHUNK):
        lhs = hT[:, n : n + 1]
        st, fin = n == 0, n == NCHUNK - 1
        wqt = wq[n // 4][:, n % 4]
        nc.tensor.matmul(mixedA, lhs, wqt[:, 0:512], start=st, stop=fin)
        nc.tensor.matmul(mixedB, lhs, wqt[:, 512:1024], start=st, stop=fin)
        nc.tensor.matmul(ba_ps, lhs, wba[:, n], start=st, stop=fin)

    # ---- phase 2: mixed row -> columns, conv update -----------------------
    mixed_sb = spool.tile([1, 1024], F32)
    nc.scalar.copy(mixed_sb[:, 0:512], mixedA)
    nc.vector.tensor_copy(mixed_sb[:, 512:1024], mixedB)
    mcol_ps = pp.tile([P, 8], F32, tag="bankE")
    for g in range(8):
        nc.tensor.matmul(
            mcol_ps[:, g : g + 1],
            mixed_sb[:, g * P : (g + 1) * P],
            one1,
            start=True,
            stop=True,
        )
    mcol = spool.tile([P, 8], F32)
    nc.vector.tensor_copy(mcol, mcol_ps)

    # new conv state = [s1 s2 s3 mixed]; conv_out = silu(sum_k ncs_k * w_k)
    ncs = spool.tile([P, 8, KCONV], F32)
    nc.vector.tensor_copy(ncs[:, :, 0:3], cs[:, :, 1:4])
    nc.scalar.copy(ncs[:, :, 3], mcol)
    nc.gpsimd.dma_start(out=d["ncs"], in_=ncs.rearrange("p g k -> p (g k)"))
    prod = spool.tile([P, 8, KCONV], F32)
    nc.vector.tensor_mul(prod, ncs, cw)
    acc = spool.tile([P, 8], F32)
    nc.vector.reduce_sum(acc, prod, axis=mybir.AxisListType.X)
    conv_sg = spool.tile([P, 8], F32)
    nc.scalar.activation(conv_sg, acc, mybir.ActivationFunctionType.Sigmoid)
    conv = spool.tile([P, 8], F32)
    nc.vector.tensor_mul(conv, acc, conv_sg)

    # v heads as rows (col -> row via matmul with identity)
    vr_ps = pp.tile([1, 512], F32, tag="bankE")
    for h in range(NH):
        nc.tensor.matmul(
            vr_ps[:, hs(h)], conv[:, 4 + h : 5 + h], id128, start=True, stop=True
        )
    v_sb = spool.tile([1, 512], F32)
    nc.vector.tensor_copy(v_sb, vr_ps)

    # ---- phase 3: per-head scalars ----------------------------------------
    # sc layout: [0:4] = l2norm scales for q0 q1 k0 k1 (q scaled by 1/sqrt(dk))
    #            [4:8] = exp(g) per v-head, [8:12] = beta per v-head
    sc = spool.tile([1, 12], F32)
    sqqk = spool.tile([P, 4], F32)
    nc.vector.tensor_mul(sqqk, conv[:, 0:4], conv[:, 0:4])
    ssq_ps = pp.tile([1, 4], F32, tag="bankF")
    nc.tensor.matmul(ssq_ps, ones_col, sqqk, start=True, stop=True)
    sd = spool.tile([1, 4], F32)
    nc.scalar.activation(sd, ssq_ps, mybir.ActivationFunctionType.Sqrt, bias=eps1)
    rq = spool.tile([1, 4], F32)
    nc.vector.reciprocal(rq, sd)
    nc.vector.tensor_mul(sc[:, 0:4], rq, qksc)

    ad = spool.tile([1, 4], F32)
    nc.vector.tensor_add(ad, ba_ps[:, 4:8], dtb[:, 0:4])
    exa = spool.tile([1, 4], F32)
    nc.scalar.activation(exa, ad, mybir.ActivationFunctionType.Exp)
    sp_t = spool.tile([1, 4], F32)
    nc.scalar.activation(sp_t, exa, mybir.ActivationFunctionType.Ln, bias=1.0)
    ea = spool.tile([1, 4], F32)
    nc.scalar.activation(ea, dtb[:, 4:8], mybir.ActivationFunctionType.Exp)
    pg = spool.tile([1, 4], F32)
    nc.vector.tensor_mul(pg, sp_t, ea)
    nc.scalar.activation(
        sc[:, 4:8], pg, mybir.ActivationFunctionType.Exp, scale=-1.0
    )
    nc.scalar.activation(
        sc[:, 8:12], ba_ps[:, 0:4], mybir.ActivationFunctionType.Sigmoid
    )

    # broadcast the 8 column scales across partitions: bc = ones (x) sc[0:8]
    bc_ps = pp.tile([P, 8], F32, tag="bankF")
    nc.tensor.matmul(bc_ps, ones_row, sc[:, 0:8], start=True, stop=True)
    bc = spool.tile([P, 8], F32)
    nc.vector.tensor_copy(bc, bc_ps)

    # scaled q/k columns
    qks = spool.tile([P, 4], F32)
    nc.vector.tensor_mul(qks, conv[:, 0:4], bc[:, 0:4])

    # k heads as rows (for the rank-1 update)
    kr_ps = pp.tile([1, NKH * P], F32, tag="bankE")
    for kh in range(NKH):
        nc.tensor.matmul(
            kr_ps[:, hs(kh)], qks[:, 2 + kh : 3 + kh], id128, start=True, stop=True
        )
    kr = spool.tile([1, NKH * P], F32)
    nc.vector.tensor_copy(kr, kr_ps)

    # ---- phase 4: gated delta rule ----------------------------------------
    recg = []
    for h in range(NH):
        t = spool.tile([P, P], F32, tag=f"recg{h}")
        nc.vector.tensor_scalar_mul(t, rec[:, h], bc[:, 4 + h : 5 + h])
        recg.append(t)

    kv_ps = pp.tile([1, 512], F32, tag="bankA")
    for h in range(NH):
        nc.tensor.matmul(
            kv_ps[:, hs(h)], qks[:, 2 + h // 2 : 3 + h // 2], recg[h],
            start=True, stop=True,
        )
    dm = spool.tile([1, 512], F32)
    nc.vector.tensor_sub(dm, v_sb, kv_ps)
    delta = spool.tile([1, 512], F32)
    for h in range(NH):
        nc.vector.tensor_scalar_mul(delta[:, hs(h)], dm[:, hs(h)], sc[:, 8 + h : 9 + h])

    core_ps = pp.tile([1, 512], F32, tag="bankB")
    rec_new = []
    for h in range(NH):
        outer = pp2.tile([P, P], F32, tag="outer")
        nc.tensor.matmul(outer, kr[:, hs(h // 2)], delta[:, hs(h)], start=True, stop=True)
        rn = spool.tile([P, P], F32, tag=f"recnew{h}")
        nc.vector.tensor_add(rn, recg[h], outer)
        rec_new.append(rn)
        nc.gpsimd.dma_start(out=d["rec_out"][h], in_=rn)
        nc.tensor.matmul(
            core_ps[:, hs(h)], qks[:, h // 2 : h // 2 + 1], rn, start=True, stop=True
        )

    # ---- z matvec (emitted after delta-rule so the PE's static order
    # doesn't stall the recurrence on the later-arriving W_z DMA) -----------
    z_ps = pp.tile([1, 512], F32, tag="bankC")
    for n in range(NCHUNK):
        nc.tensor.matmul(
            z_ps, hT[:, n : n + 1], wz[n // 8][:, n % 8],
            start=(n == 0), stop=(n == NCHUNK - 1),
        )
    zsg = spool.tile([1, 512], F32)
    nc.scalar.activation(zsg, z_ps, mybir.ActivationFunctionType.Sigmoid)
    zs = spool.tile([1, 512], F32)
    nc.vector.tensor_mul(zs, z_ps, zsg)
    zsn = spool.tile([1, 512], F32)
    nc.vector.tensor_mul(zsn, zs, nw4)

    # ---- phase 5: gated RMSNorm + output projection -----------------------
    core_sb = spool.tile([1, 512], F32)
    nc.vector.tensor_copy(core_sb, core_ps)
    sqc = spool.tile([1, 512], F32)
    nc.vector.tensor_mul(sqc, core_sb, core_sb)
    ssqc = spool.tile([1, 4], F32)
    nc.vector.reduce_sum(
        ssqc, sqc.rearrange("o (h p) -> o h p", p=P), axis=mybir.AxisListType.X
    )
    sdc = spool.tile([1, 4], F32)
    nc.scalar.activation(
        sdc, ssqc, mybir.ActivationFunctionType.Sqrt, bias=eps1, scale=1.0 / P
    )
    rsv = spool.tile([1, 4], F32)
    nc.vector.reciprocal(rsv, sdc)

    out_row = spool.tile([1, 512], F32)
    for h in range(NH):
        nc.vector.tensor_scalar_mul(
            out_row[:, hs(h)], core_sb[:, hs(h)], rsv[:, h : h + 1]
        )
    nc.vector.tensor_mul(out_row, out_row, zsn)

    oc_ps = pp.tile([P, 4], F32, tag="bankD")
    for oc in range(NH):
        nc.tensor.matmul(
            oc_ps[:, oc : oc + 1], out_row[:, hs(oc)], one1, start=True, stop=True
        )
    oc_sb = spool.tile([P, 4], F32)
    nc.vector.tensor_copy(oc_sb, oc_ps)

    _hid_tags = ["bankE", "bankF", "outer", "bankC"]
    hid_ps = [
        (pp2 if _hid_tags[j] == "outer" else pp).tile(
            [1, 512], F32, tag=_hid_tags[j], name=f"hid_ps{j}"
        )
        for j in range(4)
    ]
    for oc in range(4):
        for j in range(4):
            nc.tensor.matmul(
                hid_ps[j],
                oc_sb[:, oc : oc + 1],
                wo[oc][:, j * 512 : (j + 1) * 512],
                start=(oc == 0),
                stop=(oc == 3),
            )
    hid_sb = spool.tile([1, 2048], F32)
    for j in range(4):
        if j % 2 == 0:
            nc.vector.tensor_copy(hid_sb[:, j * 512 : (j + 1) * 512], hid_ps[j])
        else:
            nc.scalar.copy(hid_sb[:, j * 512 : (j + 1) * 512], hid_ps[j])
    nc.gpsimd.dma_start(out=d["hid"], in_=hid_sb)


def build_nc(reps: int = 1) -> bass.Bass:
    nc = bacc.Bacc(
        "TRN2",
        target_bir_lowering=False,
        debug=False,
        enable_asserts=False,
        num_devices=NCORES,
    )
    d = {}
    ins = {
        "hT": (P, NCHUNK),
        "wqkvT": (H, ROWS_PER_CORE),
        "wzT": (H, 512),
        "wbaT": (P, P),
        "woT": (512, H),
        "cs": (P, 8 * KCONV),
        "cw": (P, 8 * KCONV),
        "rec": (NH, P, P),
        "dtba": (1, 8),
        "nw4": (1, 512),
        "qksc": (1, 4),
    }
    outs = {
        "hid": (1, H),
        "ncs": (P, 8 * KCONV),
        "rec_out": (NH, P, P),
    }
    for name, shape in ins.items():
        d[name] = nc.dram_tensor(name, shape, F32, kind="ExternalInput").ap()
    for name, shape in outs.items():
        d[name] = nc.dram_tensor(name, shape, F32, kind="ExternalOutput").ap()
    with tile.TileContext(nc) as tc, ExitStack() as ctx:
        pools = (
            ctx.enter_context(tc.tile_pool(name="weights", bufs=1)),
            ctx.enter_context(tc.tile_pool(name="consts", bufs=1)),
            ctx.enter_context(tc.tile_pool(name="work", bufs=1)),
            ctx.enter_context(tc.tile_pool(name="ps", bufs=1, space="PSUM")),
            ctx.enter_context(tc.tile_pool(name="ps2", bufs=2, space="PSUM")),
        )
        for _rep in range(reps):
            _build_body(ctx, tc, d, pools)
    nc.compile()
    return nc


_NC_CACHE = None


def _get_nc() -> bass.Bass:
    global _NC_CACHE
    if _NC_CACHE is None:
        _NC_CACHE = build_nc()
    return _NC_CACHE


def make_in_maps(inputs: dict) -> list[dict]:
    f = lambda a: np.ascontiguousarray(np.asarray(a, dtype=np.float32))
    h = f(inputs["hidden_in"]).reshape(H)
    W_qkv = f(inputs["W_qkv"])
    W_z = f(inputs["W_z"])
    W_b = f(inputs["W_b"])
    W_a = f(inputs["W_a"])
    W_out = f(inputs["W_out"])
    conv_state = f(inputs["conv_state"])[0]
    conv_w = f(inputs["conv_w"])
    rec_state = f(inputs["rec_state"])[0]
    dt_bias = f(inputs["dt_bias"])
    A_log = f(inputs["A_log"])
    norm_w = f(inputs["norm_w"])

    hT = np.ascontiguousarray(h.reshape(NCHUNK, P).T)
    nw4 = np.ascontiguousarray(np.tile(norm_w, NH).reshape(1, 512))
    qksc = np.array([[RQK, RQK, 1.0, 1.0]], dtype=np.float32)

    in_maps = []
    for r in range(NCORES):
        rows = _core_rows(r)
        vsl = slice(512 * r, 512 * r + 512)
        hsl = slice(4 * r, 4 * r + 4)
        wba = np.concatenate([W_b[hsl], W_a[hsl]], axis=0)  # (8, 2048)
        in_maps.append(
            {
                "hT": hT,
                "wqkvT": np.ascontiguousarray(W_qkv[rows].T),
                "wzT": np.ascontiguousarray(W_z[vsl].T),
                "wbaT": np.ascontiguousarray(
                    wba.reshape(8, NCHUNK, P).transpose(2, 1, 0).reshape(P, P)
                ),
                "woT": np.ascontiguousarray(W_out[:, vsl].T),
                "cs": np.ascontiguousarray(
                    conv_state[rows].reshape(8, P, KCONV).transpose(1, 0, 2)
                    .reshape(P, 8 * KCONV)
                ),
                "cw": np.ascontiguousarray(
                    conv_w[rows].reshape(8, P, KCONV).transpose(1, 0, 2)
                    .reshape(P, 8 * KCONV)
                ),
                "rec": np.ascontiguousarray(rec_state[hsl]),
                "dtba": np.concatenate([dt_bias[hsl], A_log[hsl]]).reshape(1, 8),
                "nw4": nw4,
                "qksc": qksc,
            }
        )
    return in_maps


def assemble(results: list[dict]) -> tuple:
    hid = np.zeros(H, dtype=np.float32)
    ncs_full = np.zeros((2 * H + 4096, KCONV), dtype=np.float32)
    rec_full = np.zeros((32, P, P), dtype=np.float32)
    for r in range(NCORES):
        res = results[r]
        hid += res["hid"].reshape(H)
        rows = _core_rows(r)
        ncs_full[rows] = (
            res["ncs"].reshape(P, 8, KCONV).transpose(1, 0, 2).reshape(1024, KCONV)
        )
        rec_full[4 * r : 4 * r + 4] = res["rec_out"].reshape(NH, P, P)
    return (
        hid.reshape(1, 1, H),
        ncs_full[None],
        rec_full[None],
    )


def kernel(**inputs):
    nc = _get_nc()
    in_maps = make_in_maps(inputs)
    res = run_bass_kernel_spmd(nc, in_maps, core_ids=list(range(NCORES)))
    return assemble(res.results)
